# revision 6
# baseline (speedup 1.0000x reference)
"""Trainium2 Bass kernel for nn_BaseSociety (moe_routing).

Math (reference.py): 4 sequential compute_hard steps on h [B,H]:
  proj = h @ Wr[e]  -> mu, logstd       (per expert)
  kl_e = sum(mu^2 + exp(2 ls) - 1 - 2 ls)   (router score)
  winner = argmax_e kl
  z = mu_w + exp(ls_w) * eps
  h += z @ Wc[winner]
Steps: enc (E=1), comp x2 (E=8, shared weights), dec (E=1).

Sharding: data-parallel over batch across the 8 cores (routing is
per-sample independent, so there is zero cross-core communication).
Each core processes B/8 = 512 samples with full weights.

Layout inside a core: h is kept feature-major (hT: [128 part, 16 hchunk,
512 samples]) so it can serve as the matmul stationary operand; proj is
produced sample-major ([128 samples, 512]) where the KL reduction is a
free-dim accumulate fused into ScalarE activations, and the winner mask
is a per-partition scalar.  Winner dispatch for the Wc matmul is done by
transposing z with diag(mask) instead of the identity, which masks and
transposes in one TensorE op.  Z candidates are staged through DRAM to
keep SBUF under budget.
"""

import numpy as np

B, H, Z, E = 4096, 2048, 512, 8
NCORES = 8
BL = B // NCORES          # 512 samples per core
NS = BL // 128            # 4 sample tiles per core
HC = H // 128             # 16 h chunks
ZC = Z // 128             # 4 z chunks

# Matmul precision: "f32r" is 4x faster than "f32" on the PE but rounds
# operands to ~tf32 (1.6e-4).  The router argmax needs accuracy; set by
# experiment.
WR_DTYPE = "f32r"
WC_DTYPE = "f32r"

_CACHE = {}


# --------------------------------------------------------------------------
# walrus in this container rejects >1 sem-wait per instruction
# ("Too many sync wait commands").  Split excess waits onto same-engine
# NoOps inserted right before the offending instruction.
def _split_multiwait(nc, limit=1):
    import concourse.mybir as mybir
    import bass_rust as _br

    ctr = 0
    for f in nc.m.functions:
        for b in f.blocks:
            insts = b.instructions
            new = []
            changed = False
            for inst in insts:
                si = inst.sync_info
                if si is not None and len(si.on_wait) > limit:
                    waits = list(si.on_wait)
                    keep = waits[-limit:]
                    excess = waits[: len(waits) - limit]
                    for i in range(0, len(excess), limit):
                        nop = mybir.InstNoOp(
                            name=f"__waitsplit_{ctr}", ins=[], outs=[])
                        ctr += 1
                        nop.engine = inst.engine
                        nop.sync_info = _br.SyncInfo(
                            on_wait=excess[i:i + limit], on_update=[])
                        nc.register_instruction(nop, overwrite=True)
                        new.append(nop)
                    inst.sync_info = _br.SyncInfo(
                        on_wait=keep, on_update=list(si.on_update))
                    changed = True
                new.append(inst)
            if changed:
                b.instructions = new


def _build(named_scopes=False):
    import concourse.bass as bass
    import concourse.mybir as mybir
    import concourse.tile as tile
    from concourse.masks import make_identity
    from contextlib import ExitStack, nullcontext

    D = mybir.dt
    f32 = D.float32
    wr_dt = D.float32r if WR_DTYPE == "f32r" else D.float32
    wc_dt = D.float32r if WC_DTYPE == "f32r" else D.float32
    AF = mybir.ActivationFunctionType
    OP = mybir.AluOpType
    AX = mybir.AxisListType

    nc = bass.Bass()

    x_d = nc.dram_tensor("x", [BL, H], f32, kind="ExternalInput")
    wr_enc = nc.dram_tensor("wr_enc", [H, 2 * Z], wr_dt, kind="ExternalInput")
    wc_enc = nc.dram_tensor("wc_enc", [Z, H], wc_dt, kind="ExternalInput")
    wr_comp = nc.dram_tensor("wr_comp", [E, H, 2 * Z], wr_dt, kind="ExternalInput")
    wc_comp = nc.dram_tensor("wc_comp", [E, Z, H], wc_dt, kind="ExternalInput")
    wr_dec = nc.dram_tensor("wr_dec", [H, 2 * Z], wr_dt, kind="ExternalInput")
    wc_dec = nc.dram_tensor("wc_dec", [Z, H], wc_dt, kind="ExternalInput")
    eps_d = {
        "enc": nc.dram_tensor("eps_enc", [BL, Z], f32, kind="ExternalInput"),
        "c1": nc.dram_tensor("eps1", [BL, Z], f32, kind="ExternalInput"),
        "c2": nc.dram_tensor("eps2", [BL, Z], f32, kind="ExternalInput"),
        "dec": nc.dram_tensor("eps_dec", [BL, Z], f32, kind="ExternalInput"),
    }
    rev8_d = nc.dram_tensor("rev8", [128, E], f32, kind="ExternalInput")
    out_d = nc.dram_tensor("out", [BL, H], f32, kind="ExternalOutput")

    with tile.TileContext(nc) as tc, ExitStack() as ctx:
        scope = (lambda name: nc.named_scope(name)) if named_scopes else (
            lambda name: nullcontext())

        persist = ctx.enter_context(tc.tile_pool(name="persist", bufs=1))
        wr_pool = ctx.enter_context(tc.tile_pool(name="wr", bufs=6))
        wc_pool = ctx.enter_context(tc.tile_pool(name="wc", bufs=6))
        eps_pool = ctx.enter_context(tc.tile_pool(name="eps", bufs=1))
        zst_pool = ctx.enter_context(tc.tile_pool(name="zst", bufs=4))
        zmt_pool = ctx.enter_context(tc.tile_pool(name="zmt", bufs=1))
        work = ctx.enter_context(tc.tile_pool(name="work", bufs=4))
        mu_pool = ctx.enter_context(tc.tile_pool(name="mu", bufs=6))
        small = ctx.enter_context(tc.tile_pool(name="small", bufs=8))
        kls_pool = ctx.enter_context(tc.tile_pool(name="klsp", bufs=1))
        ps = ctx.enter_context(tc.tile_pool(name="ps", bufs=8, space="PSUM"))
        zdram = ctx.enter_context(tc.tile_pool(name="zdram", bufs=1, space="DRAM"))

        # persistent tiles
        hT = persist.tile([128, HC, BL], wr_dt, tag="hT")
        ident = persist.tile([128, 128], f32, tag="ident")
        make_identity(nc, ident)
        ident_r = persist.tile([128, 128], wr_dt, tag="ident_r")
        nc.vector.tensor_copy(ident_r, ident)
        rev8 = persist.tile([128, E], f32, tag="rev8")
        nc.sync.dma_start(out=rev8, in_=rev8_d[:, :])

        # ---- load x and transpose into hT --------------------------------
        with scope("load_x"):
            for s in range(NS):
                for hg in range(HC // 4):
                    xs = work.tile([128, 512], f32, tag="xs")
                    nc.sync.dma_start(
                        out=xs,
                        in_=x_d[s * 128:(s + 1) * 128, hg * 512:(hg + 1) * 512])
                    for hi in range(4):
                        hc = hg * 4 + hi
                        pt = ps.tile([128, 128], f32, tag="ps")
                        nc.tensor.transpose(
                            pt, xs[:, hi * 128:(hi + 1) * 128], ident)
                        nc.scalar.activation(
                            hT[:, hc, s * 128:(s + 1) * 128], pt, AF.Copy)

        # ---- one compute_hard step ---------------------------------------
        def step(wr_d, wc_d, eps_dram, nexp, tag):
            # phase A: router matmuls + z candidates (+ KL for nexp>1)
            eps_t = []
            for s in range(NS):
                et = eps_pool.tile([128, Z], f32, tag=f"eps{s}")
                nc.sync.dma_start(out=et, in_=eps_dram[s * 128:(s + 1) * 128, :])
                eps_t.append(et)

            kls = [kls_pool.tile([128, E], f32, tag=f"kls{i}", name=f"kls_{tag}_{i}")
                   for i in range(NS)] if nexp > 1 else None
            z_dram = {}
            with scope(f"{tag}_router"):
                for e in range(nexp):
                    pmu = [ps.tile([128, Z], f32, tag="ps", name=f"pmu_{tag}_{e}_{i}") for i in range(NS)]
                    for hc in range(HC):
                        w = wr_pool.tile([128, Z], wr_dt, tag="wr")
                        if nexp > 1:
                            src = wr_d[e, hc * 128:(hc + 1) * 128, 0:Z]
                        else:
                            src = wr_d[hc * 128:(hc + 1) * 128, 0:Z]
                        nc.sync.dma_start(out=w, in_=src)
                        for s in range(NS):
                            nc.tensor.matmul(
                                pmu[s], hT[:, hc, s * 128:(s + 1) * 128], w,
                                start=(hc == 0), stop=(hc == HC - 1))
                    # evacuate mu (+ running sum(mu^2) for the router)
                    mu_sb = []
                    for s in range(NS):
                        m = mu_pool.tile([128, Z], f32, tag="mu_sb")
                        nc.scalar.activation(m, pmu[s], AF.Copy)
                        a_mu2 = None
                        if nexp > 1:
                            sq = work.tile([128, Z], f32, tag="scratch")
                            a_mu2 = small.tile([128, 1], f32, tag="a_mu2")
                            nc.scalar.activation(sq, pmu[s], AF.Square,
                                                 scale=0.7071067811865476,
                                                 accum_out=a_mu2)
                        mu_sb.append((m, a_mu2))
                    pls = [ps.tile([128, Z], f32, tag="ps", name=f"pls_{tag}_{e}_{i}") for i in range(NS)]
                    for hc in range(HC):
                        w = wr_pool.tile([128, Z], wr_dt, tag="wr")
                        if nexp > 1:
                            src = wr_d[e, hc * 128:(hc + 1) * 128, Z:2 * Z]
                        else:
                            src = wr_d[hc * 128:(hc + 1) * 128, Z:2 * Z]
                        nc.sync.dma_start(out=w, in_=src)
                        for s in range(NS):
                            nc.tensor.matmul(
                                pls[s], hT[:, hc, s * 128:(s + 1) * 128], w,
                                start=(hc == 0), stop=(hc == HC - 1))
                    for s in range(NS):
                        mu, a_mu2 = mu_sb[s]
                        ex = work.tile([128, Z], f32, tag="ex")
                        nc.scalar.activation(ex, pls[s], AF.Exp)
                        if nexp > 1:
                            a_ls = small.tile([128, 1], f32, tag="a_ls")
                            sc1 = work.tile([128, Z], f32, tag="scratch")
                            nc.scalar.activation(sc1, pls[s], AF.Copy,
                                                 accum_out=a_ls)
                            a_e2 = small.tile([128, 1], f32, tag="a_e2")
                            sc2 = work.tile([128, Z], f32, tag="scratch")
                            nc.scalar.activation(sc2, ex, AF.Square,
                                                 scale=0.7071067811865476,
                                                 accum_out=a_e2)
                            # kl' = a_mu2 + a_e2 - 2 a_ls (argmax-equivalent)
                            t1 = small.tile([128, 1], f32, tag="t1")
                            nc.vector.tensor_add(t1, a_mu2, a_e2)
                            nc.vector.tensor_scalar(
                                out=kls[s][:, e:e + 1], in0=a_ls,
                                scalar1=-1.0, scalar2=t1,
                                op0=OP.mult, op1=OP.add)
                        exeps = work.tile([128, Z], f32, tag="exeps")
                        nc.vector.tensor_mul(exeps, ex, eps_t[s])
                        zt = work.tile([128, Z], f32, tag="zt")
                        nc.vector.tensor_add(zt, mu, exeps)
                        zd = zdram.tile([128, Z], f32, tag=f"z{e}_{s}")
                        nc.sync.dma_start(out=zd, in_=zt)
                        z_dram[(e, s)] = zd

            # phase B/C: winner mask, masked transpose into zmT
            zmT = [zmt_pool.tile([128, ZC, BL], wc_dt, tag=f"zmT{i}", name=f"zmT_{tag}_{i}")
                   for i in range(nexp)]
            with scope(f"{tag}_select"):
                for s in range(NS):
                    fw = None
                    if nexp > 1:
                        m = small.tile([128, 1], f32, tag="klmax")
                        nc.vector.reduce_max(m, kls[s], axis=AX.X)
                        eq = small.tile([128, E], f32, tag="eq")
                        nc.vector.tensor_scalar(
                            out=eq, in0=kls[s], scalar1=m, scalar2=None,
                            op0=OP.is_equal)
                        # first-index tie-break (matches jnp.argmax): the
                        # first winner has the largest eq*rev8 = 8-e > 0.
                        tsel = small.tile([128, E], f32, tag="tsel")
                        nc.vector.tensor_mul(tsel, eq, rev8)
                        m2 = small.tile([128, 1], f32, tag="klmax2")
                        nc.vector.reduce_max(m2, tsel, axis=AX.X)
                        fw = small.tile([128, E], f32, tag="fw")
                        nc.vector.tensor_scalar(
                            out=fw, in0=tsel, scalar1=m2, scalar2=None,
                            op0=OP.is_equal)
                    for e in range(nexp):
                        zs = zst_pool.tile([128, Z], f32, tag="zstage")
                        nc.sync.dma_start(out=zs, in_=z_dram[(e, s)])
                        if nexp > 1:
                            dg = work.tile([128, 128], f32, tag="diag")
                            nc.vector.tensor_scalar(
                                out=dg, in0=ident, scalar1=fw[:, e:e + 1],
                                scalar2=None, op0=OP.mult)
                        else:
                            dg = ident
                        for zc in range(ZC):
                            pt = ps.tile([128, 128], f32, tag="ps")
                            nc.tensor.transpose(
                                pt, zs[:, zc * 128:(zc + 1) * 128], dg)
                            nc.scalar.activation(
                                zmT[e][:, zc, s * 128:(s + 1) * 128], pt,
                                AF.Copy)

            # phase D: h += sum_e zm_e @ Wc[e]
            with scope(f"{tag}_expert_out"):
                for hg in range(HC // 4):
                    ph = [ps.tile([128, 512], f32, tag="ps", name=f"ph_{tag}_{hg}_{i}") for i in range(4)]
                    for e in range(nexp):
                        for zc in range(ZC):
                            w = wc_pool.tile([128, 512], wc_dt, tag="wc")
                            if nexp > 1:
                                src = wc_d[e, zc * 128:(zc + 1) * 128,
                                           hg * 512:(hg + 1) * 512]
                            else:
                                src = wc_d[zc * 128:(zc + 1) * 128,
                                           hg * 512:(hg + 1) * 512]
                            nc.sync.dma_start(out=w, in_=src)
                            for hi in range(4):
                                nc.tensor.matmul(
                                    ph[hi], w[:, hi * 128:(hi + 1) * 128],
                                    zmT[e][:, zc, :],
                                    start=(e == 0 and zc == 0),
                                    stop=(e == nexp - 1 and zc == ZC - 1))
                    for hi in range(4):
                        hc = hg * 4 + hi
                        nc.vector.tensor_add(hT[:, hc, :], hT[:, hc, :], ph[hi])

        step(wr_enc, wc_enc, eps_d["enc"], 1, "enc")
        step(wr_comp, wc_comp, eps_d["c1"], E, "c1")
        step(wr_comp, wc_comp, eps_d["c2"], E, "c2")
        step(wr_dec, wc_dec, eps_d["dec"], 1, "dec")

        # ---- final transpose back to sample-major and store --------------
        with scope("store"):
            for s in range(NS):
                for hg in range(HC // 4):
                    ot = work.tile([128, 512], f32, tag="xs")
                    for hi in range(4):
                        hc = hg * 4 + hi
                        pt = ps.tile([128, 128], wr_dt, tag="ps")
                        nc.tensor.transpose(
                            pt, hT[:, hc, s * 128:(s + 1) * 128], ident_r)
                        nc.scalar.activation(
                            ot[:, hi * 128:(hi + 1) * 128], pt, AF.Copy)
                    nc.sync.dma_start(
                        out=out_d[s * 128:(s + 1) * 128,
                                  hg * 512:(hg + 1) * 512],
                        in_=ot)

    _split_multiwait(nc, limit=1)
    return nc


def get_program(named_scopes=False):
    key = (WR_DTYPE, WC_DTYPE, named_scopes)
    if key not in _CACHE:
        _CACHE[key] = _build(named_scopes=named_scopes)
    return _CACHE[key]


def make_in_maps(inputs):
    x = np.ascontiguousarray(inputs["x"], dtype=np.float32)
    shared = {
        "wr_enc": np.ascontiguousarray(inputs["Wr_enc"][0], np.float32),
        "wc_enc": np.ascontiguousarray(inputs["Wc_enc"][0], np.float32),
        "wr_comp": np.ascontiguousarray(inputs["Wr_comp"], np.float32),
        "wc_comp": np.ascontiguousarray(inputs["Wc_comp"], np.float32),
        "wr_dec": np.ascontiguousarray(inputs["Wr_dec"][0], np.float32),
        "wc_dec": np.ascontiguousarray(inputs["Wc_dec"][0], np.float32),
        "rev8": np.tile(np.arange(E, 0, -1, dtype=np.float32), (128, 1)),
    }
    maps = []
    for c in range(NCORES):
        sl = slice(c * BL, (c + 1) * BL)
        m = dict(shared)
        m["x"] = x[sl]
        m["eps_enc"] = np.ascontiguousarray(inputs["eps_enc"][sl], np.float32)
        m["eps1"] = np.ascontiguousarray(inputs["eps1"][sl], np.float32)
        m["eps2"] = np.ascontiguousarray(inputs["eps2"][sl], np.float32)
        m["eps_dec"] = np.ascontiguousarray(inputs["eps_dec"][sl], np.float32)
        maps.append(m)
    return maps


def kernel(**inputs):
    from concourse.bass_utils import run_bass_kernel_spmd

    nc = get_program()
    res = run_bass_kernel_spmd(nc, make_in_maps(inputs),
                               core_ids=list(range(NCORES)))
    out = np.concatenate([res.results[c]["out"] for c in range(NCORES)], axis=0)
    return out.astype(np.float32)


# revision 7
# speedup vs baseline: 1.2033x; 1.2033x over previous
"""Trainium2 Bass kernel for nn_BaseSociety (moe_routing).

Math (reference.py): 4 sequential compute_hard steps on h [B,H]:
  proj = h @ Wr[e]  -> mu, logstd       (per expert)
  kl_e = sum(mu^2 + exp(2 ls) - 1 - 2 ls)   (router score)
  winner = argmax_e kl
  z = mu_w + exp(ls_w) * eps
  h += z @ Wc[winner]
Steps: enc (E=1), comp x2 (E=8, shared weights), dec (E=1).

Sharding: data-parallel over batch across the 8 cores (routing is
per-sample independent, so there is zero cross-core communication).
Each core processes B/8 = 512 samples with full weights.

Layout inside a core: h is kept feature-major (hT: [128 part, 16 hchunk,
512 samples]) so it can serve as the matmul stationary operand; proj is
produced sample-major ([128 samples, 512]) where the KL reduction is a
free-dim accumulate fused into ScalarE activations, and the winner mask
is a per-partition scalar.  Winner dispatch for the Wc matmul is done by
transposing z with diag(mask) instead of the identity, which masks and
transposes in one TensorE op.  Z candidates are staged through DRAM to
keep SBUF under budget.
"""

import numpy as np

B, H, Z, E = 4096, 2048, 512, 8
NCORES = 8
BL = B // NCORES          # 512 samples per core
NS = BL // 128            # 4 sample tiles per core
HC = H // 128             # 16 h chunks
ZC = Z // 128             # 4 z chunks

# Matmul precision: "f32r" is 4x faster than "f32" on the PE but rounds
# operands to ~tf32 (1.6e-4).  The router argmax needs accuracy; set by
# experiment.
WR_DTYPE = "bf16"
WC_DTYPE = "bf16"

_CACHE = {}


# --------------------------------------------------------------------------
# walrus in this container rejects >1 sem-wait per instruction
# ("Too many sync wait commands").  Split excess waits onto same-engine
# NoOps inserted right before the offending instruction.
def _split_multiwait(nc, limit=1):
    import concourse.mybir as mybir
    import bass_rust as _br

    ctr = 0
    for f in nc.m.functions:
        for b in f.blocks:
            insts = b.instructions
            new = []
            changed = False
            for inst in insts:
                si = inst.sync_info
                if si is not None and len(si.on_wait) > limit:
                    waits = list(si.on_wait)
                    keep = waits[-limit:]
                    excess = waits[: len(waits) - limit]
                    for i in range(0, len(excess), limit):
                        nop = mybir.InstNoOp(
                            name=f"__waitsplit_{ctr}", ins=[], outs=[])
                        ctr += 1
                        nop.engine = inst.engine
                        nop.sync_info = _br.SyncInfo(
                            on_wait=excess[i:i + limit], on_update=[])
                        nc.register_instruction(nop, overwrite=True)
                        new.append(nop)
                    inst.sync_info = _br.SyncInfo(
                        on_wait=keep, on_update=list(si.on_update))
                    changed = True
                new.append(inst)
            if changed:
                b.instructions = new


def _build(named_scopes=False):
    import concourse.bass as bass
    import concourse.mybir as mybir
    import concourse.tile as tile
    from concourse.masks import make_identity
    from contextlib import ExitStack, nullcontext

    D = mybir.dt
    f32 = D.float32
    dtmap = {"f32": D.float32, "f32r": D.float32r, "bf16": D.bfloat16}
    wr_dt = dtmap[WR_DTYPE]
    wc_dt = dtmap[WC_DTYPE]
    AF = mybir.ActivationFunctionType
    OP = mybir.AluOpType
    AX = mybir.AxisListType

    nc = bass.Bass()

    x_d = nc.dram_tensor("x", [BL, H], f32, kind="ExternalInput")
    wr_enc = nc.dram_tensor("wr_enc", [H, 2 * Z], wr_dt, kind="ExternalInput")
    wc_enc = nc.dram_tensor("wc_enc", [Z, H], wc_dt, kind="ExternalInput")
    wr_comp = nc.dram_tensor("wr_comp", [E, H, 2 * Z], wr_dt, kind="ExternalInput")
    wc_comp = nc.dram_tensor("wc_comp", [E, Z, H], wc_dt, kind="ExternalInput")
    wr_dec = nc.dram_tensor("wr_dec", [H, 2 * Z], wr_dt, kind="ExternalInput")
    wc_dec = nc.dram_tensor("wc_dec", [Z, H], wc_dt, kind="ExternalInput")
    eps_d = {
        "enc": nc.dram_tensor("eps_enc", [BL, Z], f32, kind="ExternalInput"),
        "c1": nc.dram_tensor("eps1", [BL, Z], f32, kind="ExternalInput"),
        "c2": nc.dram_tensor("eps2", [BL, Z], f32, kind="ExternalInput"),
        "dec": nc.dram_tensor("eps_dec", [BL, Z], f32, kind="ExternalInput"),
    }
    rev8_d = nc.dram_tensor("rev8", [128, E], f32, kind="ExternalInput")
    out_d = nc.dram_tensor("out", [BL, H], f32, kind="ExternalOutput")

    with tile.TileContext(nc) as tc, ExitStack() as ctx:
        scope = (lambda name: nc.named_scope(name)) if named_scopes else (
            lambda name: nullcontext())

        persist = ctx.enter_context(tc.tile_pool(name="persist", bufs=1))
        wr_pool = ctx.enter_context(tc.tile_pool(name="wr", bufs=6))
        wc_pool = ctx.enter_context(tc.tile_pool(name="wc", bufs=6))
        eps_pool = ctx.enter_context(tc.tile_pool(name="eps", bufs=1))
        zmt_pool = ctx.enter_context(tc.tile_pool(name="zmt", bufs=1))
        work = ctx.enter_context(tc.tile_pool(name="work", bufs=4))
        mu_pool = ctx.enter_context(tc.tile_pool(name="mu", bufs=6))
        small = ctx.enter_context(tc.tile_pool(name="small", bufs=8))
        kls_pool = ctx.enter_context(tc.tile_pool(name="klsp", bufs=1))
        ps = ctx.enter_context(tc.tile_pool(name="ps", bufs=8, space="PSUM"))
        zc_pool = ctx.enter_context(tc.tile_pool(name="zcand", bufs=1))

        # persistent tiles
        hT = persist.tile([128, HC, BL], wr_dt, tag="hT")
        ident = persist.tile([128, 128], f32, tag="ident")
        make_identity(nc, ident)
        rev8 = persist.tile([128, E], f32, tag="rev8")
        nc.sync.dma_start(out=rev8, in_=rev8_d[:, :])

        # ---- load x and transpose into hT --------------------------------
        with scope("load_x"):
            for s in range(NS):
                for hg in range(HC // 4):
                    xs = work.tile([128, 512], f32, tag="xs")
                    nc.sync.dma_start(
                        out=xs,
                        in_=x_d[s * 128:(s + 1) * 128, hg * 512:(hg + 1) * 512])
                    for hi in range(4):
                        hc = hg * 4 + hi
                        pt = ps.tile([128, 128], f32, tag="ps")
                        nc.tensor.transpose(
                            pt, xs[:, hi * 128:(hi + 1) * 128], ident)
                        nc.scalar.activation(
                            hT[:, hc, s * 128:(s + 1) * 128], pt, AF.Copy)

        # ---- one compute_hard step ---------------------------------------
        def step(wr_d, wc_d, eps_dram, nexp, tag):
            # phase A: router matmuls + z candidates (+ KL for nexp>1)
            eps_t = []
            for s in range(NS):
                et = eps_pool.tile([128, Z], f32, tag=f"eps{s}")
                nc.sync.dma_start(out=et, in_=eps_dram[s * 128:(s + 1) * 128, :])
                eps_t.append(et)

            kls = [kls_pool.tile([128, E], f32, tag=f"kls{i}", name=f"kls_{tag}_{i}")
                   for i in range(NS)] if nexp > 1 else None
            z_cand = {}
            with scope(f"{tag}_router"):
                for e in range(nexp):
                    pmu = [ps.tile([128, Z], f32, tag="ps", name=f"pmu_{tag}_{e}_{i}") for i in range(NS)]
                    for hc in range(HC):
                        w = wr_pool.tile([128, Z], wr_dt, tag="wr")
                        if nexp > 1:
                            src = wr_d[e, hc * 128:(hc + 1) * 128, 0:Z]
                        else:
                            src = wr_d[hc * 128:(hc + 1) * 128, 0:Z]
                        nc.sync.dma_start(out=w, in_=src)
                        for s in range(NS):
                            nc.tensor.matmul(
                                pmu[s], hT[:, hc, s * 128:(s + 1) * 128], w,
                                start=(hc == 0), stop=(hc == HC - 1))
                    # evacuate mu (+ running sum(mu^2) for the router)
                    mu_sb = []
                    for s in range(NS):
                        m = mu_pool.tile([128, Z], f32, tag="mu_sb")
                        nc.scalar.activation(m, pmu[s], AF.Copy)
                        a_mu2 = None
                        if nexp > 1:
                            sq = work.tile([128, Z], f32, tag="scratch")
                            a_mu2 = small.tile([128, 1], f32, tag="a_mu2")
                            nc.scalar.activation(sq, pmu[s], AF.Square,
                                                 scale=0.7071067811865476,
                                                 accum_out=a_mu2)
                        mu_sb.append((m, a_mu2))
                    pls = [ps.tile([128, Z], f32, tag="ps", name=f"pls_{tag}_{e}_{i}") for i in range(NS)]
                    for hc in range(HC):
                        w = wr_pool.tile([128, Z], wr_dt, tag="wr")
                        if nexp > 1:
                            src = wr_d[e, hc * 128:(hc + 1) * 128, Z:2 * Z]
                        else:
                            src = wr_d[hc * 128:(hc + 1) * 128, Z:2 * Z]
                        nc.sync.dma_start(out=w, in_=src)
                        for s in range(NS):
                            nc.tensor.matmul(
                                pls[s], hT[:, hc, s * 128:(s + 1) * 128], w,
                                start=(hc == 0), stop=(hc == HC - 1))
                    for s in range(NS):
                        mu, a_mu2 = mu_sb[s]
                        ex = work.tile([128, Z], f32, tag="ex")
                        nc.scalar.activation(ex, pls[s], AF.Exp)
                        if nexp > 1:
                            a_ls = small.tile([128, 1], f32, tag="a_ls")
                            sc1 = work.tile([128, Z], f32, tag="scratch")
                            nc.scalar.activation(sc1, pls[s], AF.Copy,
                                                 accum_out=a_ls)
                            a_e2 = small.tile([128, 1], f32, tag="a_e2")
                            sc2 = work.tile([128, Z], f32, tag="scratch")
                            nc.scalar.activation(sc2, ex, AF.Square,
                                                 scale=0.7071067811865476,
                                                 accum_out=a_e2)
                            # kl' = a_mu2 + a_e2 - 2 a_ls (argmax-equivalent)
                            t1 = small.tile([128, 1], f32, tag="t1")
                            nc.vector.tensor_add(t1, a_mu2, a_e2)
                            nc.vector.tensor_scalar(
                                out=kls[s][:, e:e + 1], in0=a_ls,
                                scalar1=-1.0, scalar2=t1,
                                op0=OP.mult, op1=OP.add)
                        exeps = work.tile([128, Z], f32, tag="exeps")
                        nc.vector.tensor_mul(exeps, ex, eps_t[s])
                        zt = zc_pool.tile([128, Z], f32, tag=f"z{e}_{s}",
                                          name=f"z_{tag}_{e}_{s}")
                        nc.vector.tensor_add(zt, mu, exeps)
                        z_cand[(e, s)] = zt

            # phase B/C: winner mask, masked transpose into zmT
            zmT = [zmt_pool.tile([128, ZC, BL], wc_dt, tag=f"zmT{i}", name=f"zmT_{tag}_{i}")
                   for i in range(nexp)]
            with scope(f"{tag}_select"):
                for s in range(NS):
                    fw = None
                    if nexp > 1:
                        m = small.tile([128, 1], f32, tag="klmax")
                        nc.vector.reduce_max(m, kls[s], axis=AX.X)
                        eq = small.tile([128, E], f32, tag="eq")
                        nc.vector.tensor_scalar(
                            out=eq, in0=kls[s], scalar1=m, scalar2=None,
                            op0=OP.is_equal)
                        # first-index tie-break (matches jnp.argmax): the
                        # first winner has the largest eq*rev8 = 8-e > 0.
                        tsel = small.tile([128, E], f32, tag="tsel")
                        nc.vector.tensor_mul(tsel, eq, rev8)
                        m2 = small.tile([128, 1], f32, tag="klmax2")
                        nc.vector.reduce_max(m2, tsel, axis=AX.X)
                        fw = small.tile([128, E], f32, tag="fw")
                        nc.vector.tensor_scalar(
                            out=fw, in0=tsel, scalar1=m2, scalar2=None,
                            op0=OP.is_equal)
                    for e in range(nexp):
                        zs = z_cand[(e, s)]
                        if nexp > 1:
                            dg = work.tile([128, 128], f32, tag="diag")
                            nc.vector.tensor_scalar(
                                out=dg, in0=ident, scalar1=fw[:, e:e + 1],
                                scalar2=None, op0=OP.mult)
                        else:
                            dg = ident
                        for zc in range(ZC):
                            pt = ps.tile([128, 128], f32, tag="ps")
                            nc.tensor.transpose(
                                pt, zs[:, zc * 128:(zc + 1) * 128], dg)
                            nc.scalar.activation(
                                zmT[e][:, zc, s * 128:(s + 1) * 128], pt,
                                AF.Copy)

            # phase D: h += sum_e zm_e @ Wc[e]
            with scope(f"{tag}_expert_out"):
                for hg in range(HC // 4):
                    ph = [ps.tile([128, 512], f32, tag="ps", name=f"ph_{tag}_{hg}_{i}") for i in range(4)]
                    for e in range(nexp):
                        for zc in range(ZC):
                            w = wc_pool.tile([128, 512], wc_dt, tag="wc")
                            if nexp > 1:
                                src = wc_d[e, zc * 128:(zc + 1) * 128,
                                           hg * 512:(hg + 1) * 512]
                            else:
                                src = wc_d[zc * 128:(zc + 1) * 128,
                                           hg * 512:(hg + 1) * 512]
                            nc.sync.dma_start(out=w, in_=src)
                            for hi in range(4):
                                nc.tensor.matmul(
                                    ph[hi], w[:, hi * 128:(hi + 1) * 128],
                                    zmT[e][:, zc, :],
                                    start=(e == 0 and zc == 0),
                                    stop=(e == nexp - 1 and zc == ZC - 1))
                    for hi in range(4):
                        hc = hg * 4 + hi
                        nc.vector.tensor_add(hT[:, hc, :], hT[:, hc, :], ph[hi])

        step(wr_enc, wc_enc, eps_d["enc"], 1, "enc")
        step(wr_comp, wc_comp, eps_d["c1"], E, "c1")
        step(wr_comp, wc_comp, eps_d["c2"], E, "c2")
        step(wr_dec, wc_dec, eps_d["dec"], 1, "dec")

        # ---- final transpose back to sample-major and store --------------
        with scope("store"):
            for s in range(NS):
                for hg in range(HC // 4):
                    ot = work.tile([128, 512], f32, tag="xs")
                    for hi in range(4):
                        hc = hg * 4 + hi
                        hf = work.tile([128, 128], f32, tag="hf")
                        nc.vector.tensor_copy(hf, hT[:, hc, s * 128:(s + 1) * 128])
                        pt = ps.tile([128, 128], f32, tag="ps")
                        nc.tensor.transpose(pt, hf, ident)
                        nc.scalar.activation(
                            ot[:, hi * 128:(hi + 1) * 128], pt, AF.Copy)
                    nc.sync.dma_start(
                        out=out_d[s * 128:(s + 1) * 128,
                                  hg * 512:(hg + 1) * 512],
                        in_=ot)

    _split_multiwait(nc, limit=1)
    return nc


def get_program(named_scopes=False):
    key = (WR_DTYPE, WC_DTYPE, named_scopes)
    if key not in _CACHE:
        _CACHE[key] = _build(named_scopes=named_scopes)
    return _CACHE[key]


def make_in_maps(inputs):
    import ml_dtypes
    npmap = {"f32": np.float32, "f32r": np.float32, "bf16": ml_dtypes.bfloat16}
    wr_np = npmap[WR_DTYPE]
    wc_np = npmap[WC_DTYPE]
    x = np.ascontiguousarray(inputs["x"], dtype=np.float32)
    shared = {
        "wr_enc": np.ascontiguousarray(np.asarray(inputs["Wr_enc"][0]).astype(wr_np)),
        "wc_enc": np.ascontiguousarray(np.asarray(inputs["Wc_enc"][0]).astype(wc_np)),
        "wr_comp": np.ascontiguousarray(np.asarray(inputs["Wr_comp"]).astype(wr_np)),
        "wc_comp": np.ascontiguousarray(np.asarray(inputs["Wc_comp"]).astype(wc_np)),
        "wr_dec": np.ascontiguousarray(np.asarray(inputs["Wr_dec"][0]).astype(wr_np)),
        "wc_dec": np.ascontiguousarray(np.asarray(inputs["Wc_dec"][0]).astype(wc_np)),
        "rev8": np.tile(np.arange(E, 0, -1, dtype=np.float32), (128, 1)),
    }
    maps = []
    for c in range(NCORES):
        sl = slice(c * BL, (c + 1) * BL)
        m = dict(shared)
        m["x"] = x[sl]
        m["eps_enc"] = np.ascontiguousarray(inputs["eps_enc"][sl], np.float32)
        m["eps1"] = np.ascontiguousarray(inputs["eps1"][sl], np.float32)
        m["eps2"] = np.ascontiguousarray(inputs["eps2"][sl], np.float32)
        m["eps_dec"] = np.ascontiguousarray(inputs["eps_dec"][sl], np.float32)
        maps.append(m)
    return maps


def kernel(**inputs):
    from concourse.bass_utils import run_bass_kernel_spmd

    nc = get_program()
    res = run_bass_kernel_spmd(nc, make_in_maps(inputs),
                               core_ids=list(range(NCORES)))
    out = np.concatenate([res.results[c]["out"] for c in range(NCORES)], axis=0)
    return out.astype(np.float32)


# revision 8
# speedup vs baseline: 1.2423x; 1.0324x over previous
"""Trainium2 Bass kernel for nn_BaseSociety (moe_routing).

Math (reference.py): 4 sequential compute_hard steps on h [B,H]:
  proj = h @ Wr[e]  -> mu, logstd       (per expert)
  kl_e = sum(mu^2 + exp(2 ls) - 1 - 2 ls)   (router score)
  winner = argmax_e kl
  z = mu_w + exp(ls_w) * eps
  h += z @ Wc[winner]
Steps: enc (E=1), comp x2 (E=8, shared weights), dec (E=1).

Sharding: data-parallel over batch across the 8 cores (routing is
per-sample independent, so there is zero cross-core communication).
Each core processes B/8 = 512 samples with full weights.

Layout inside a core: h is kept feature-major (hT: [128 part, 16 hchunk,
512 samples]) so it can serve as the matmul stationary operand; proj is
produced sample-major ([128 samples, 512]) where the KL reduction is a
free-dim accumulate fused into ScalarE activations, and the winner mask
is a per-partition scalar.  Winner dispatch for the Wc matmul is done by
transposing z with diag(mask) instead of the identity, which masks and
transposes in one TensorE op.  Z candidates are staged through DRAM to
keep SBUF under budget.
"""

import numpy as np

B, H, Z, E = 4096, 2048, 512, 8
NCORES = 8
BL = B // NCORES          # 512 samples per core
NS = BL // 128            # 4 sample tiles per core
HC = H // 128             # 16 h chunks
ZC = Z // 128             # 4 z chunks

# Matmul precision: "f32r" is 4x faster than "f32" on the PE but rounds
# operands to ~tf32 (1.6e-4).  The router argmax needs accuracy; set by
# experiment.
WR_DTYPE = "bf16"
WC_DTYPE = "bf16"

_CACHE = {}


# --------------------------------------------------------------------------
# walrus in this container rejects >1 sem-wait per instruction
# ("Too many sync wait commands").  Split excess waits onto same-engine
# NoOps inserted right before the offending instruction.
def _split_multiwait(nc, limit=1):
    import concourse.mybir as mybir
    import bass_rust as _br

    ctr = 0
    for f in nc.m.functions:
        for b in f.blocks:
            insts = b.instructions
            new = []
            changed = False
            for inst in insts:
                si = inst.sync_info
                if si is not None and len(si.on_wait) > limit:
                    waits = list(si.on_wait)
                    keep = waits[-limit:]
                    excess = waits[: len(waits) - limit]
                    for i in range(0, len(excess), limit):
                        nop = mybir.InstNoOp(
                            name=f"__waitsplit_{ctr}", ins=[], outs=[])
                        ctr += 1
                        nop.engine = inst.engine
                        nop.sync_info = _br.SyncInfo(
                            on_wait=excess[i:i + limit], on_update=[])
                        nc.register_instruction(nop, overwrite=True)
                        new.append(nop)
                    inst.sync_info = _br.SyncInfo(
                        on_wait=keep, on_update=list(si.on_update))
                    changed = True
                new.append(inst)
            if changed:
                b.instructions = new


def _build(named_scopes=False):
    import concourse.bass as bass
    import concourse.mybir as mybir
    import concourse.tile as tile
    from concourse.masks import make_identity
    from contextlib import ExitStack, nullcontext

    D = mybir.dt
    f32 = D.float32
    dtmap = {"f32": D.float32, "f32r": D.float32r, "bf16": D.bfloat16}
    wr_dt = dtmap[WR_DTYPE]
    wc_dt = dtmap[WC_DTYPE]
    AF = mybir.ActivationFunctionType
    OP = mybir.AluOpType
    AX = mybir.AxisListType

    nc = bass.Bass()

    x_d = nc.dram_tensor("x", [H, BL], wr_dt, kind="ExternalInput")
    wr_enc = nc.dram_tensor("wr_enc", [H, 2 * Z], wr_dt, kind="ExternalInput")
    wc_enc = nc.dram_tensor("wc_enc", [Z, H], wc_dt, kind="ExternalInput")
    wr_comp = nc.dram_tensor("wr_comp", [E, H, 2 * Z], wr_dt, kind="ExternalInput")
    wc_comp = nc.dram_tensor("wc_comp", [E, Z, H], wc_dt, kind="ExternalInput")
    wr_dec = nc.dram_tensor("wr_dec", [H, 2 * Z], wr_dt, kind="ExternalInput")
    wc_dec = nc.dram_tensor("wc_dec", [Z, H], wc_dt, kind="ExternalInput")
    eps_d = {
        "enc": nc.dram_tensor("eps_enc", [BL, Z], f32, kind="ExternalInput"),
        "c1": nc.dram_tensor("eps1", [BL, Z], f32, kind="ExternalInput"),
        "c2": nc.dram_tensor("eps2", [BL, Z], f32, kind="ExternalInput"),
        "dec": nc.dram_tensor("eps_dec", [BL, Z], f32, kind="ExternalInput"),
    }
    rev8_d = nc.dram_tensor("rev8", [128, E], f32, kind="ExternalInput")
    out_d = nc.dram_tensor("out", [H, BL], wr_dt, kind="ExternalOutput")

    with tile.TileContext(nc) as tc, ExitStack() as ctx:
        scope = (lambda name: nc.named_scope(name)) if named_scopes else (
            lambda name: nullcontext())

        persist = ctx.enter_context(tc.tile_pool(name="persist", bufs=1))
        wr_pool = ctx.enter_context(tc.tile_pool(name="wr", bufs=6))
        wc_pool = ctx.enter_context(tc.tile_pool(name="wc", bufs=6))
        eps_pool = ctx.enter_context(tc.tile_pool(name="eps", bufs=1))
        zmt_pool = ctx.enter_context(tc.tile_pool(name="zmt", bufs=1))
        work = ctx.enter_context(tc.tile_pool(name="work", bufs=4))
        mu_pool = ctx.enter_context(tc.tile_pool(name="mu", bufs=6))
        small = ctx.enter_context(tc.tile_pool(name="small", bufs=8))
        kls_pool = ctx.enter_context(tc.tile_pool(name="klsp", bufs=1))
        ps = ctx.enter_context(tc.tile_pool(name="ps", bufs=8, space="PSUM"))
        zc_pool = ctx.enter_context(tc.tile_pool(name="zcand", bufs=1))

        # persistent tiles
        hT = persist.tile([128, HC, BL], wr_dt, tag="hT")
        ident = persist.tile([128, 128], f32, tag="ident")
        make_identity(nc, ident)
        rev8 = persist.tile([128, E], f32, tag="rev8")
        nc.sync.dma_start(out=rev8, in_=rev8_d[:, :])

        # ---- load x (host supplies it feature-major) ---------------------
        with scope("load_x"):
            nc.sync.dma_start(
                out=hT, in_=x_d.rearrange("(c p) s -> p c s", p=128))

        # ---- one compute_hard step ---------------------------------------
        def step(wr_d, wc_d, eps_dram, nexp, tag):
            # phase A: router matmuls + z candidates (+ KL for nexp>1)
            eps_t = []
            for s in range(NS):
                et = eps_pool.tile([128, Z], f32, tag=f"eps{s}")
                nc.sync.dma_start(out=et, in_=eps_dram[s * 128:(s + 1) * 128, :])
                eps_t.append(et)

            kls = [kls_pool.tile([128, E], f32, tag=f"kls{i}", name=f"kls_{tag}_{i}")
                   for i in range(NS)] if nexp > 1 else None
            z_cand = {}
            with scope(f"{tag}_router"):
                for e in range(nexp):
                    pmu = [ps.tile([128, Z], f32, tag="ps", name=f"pmu_{tag}_{e}_{i}") for i in range(NS)]
                    for hc in range(HC):
                        w = wr_pool.tile([128, Z], wr_dt, tag="wr")
                        if nexp > 1:
                            src = wr_d[e, hc * 128:(hc + 1) * 128, 0:Z]
                        else:
                            src = wr_d[hc * 128:(hc + 1) * 128, 0:Z]
                        nc.sync.dma_start(out=w, in_=src)
                        for s in range(NS):
                            nc.tensor.matmul(
                                pmu[s], hT[:, hc, s * 128:(s + 1) * 128], w,
                                start=(hc == 0), stop=(hc == HC - 1))
                    # evacuate mu (+ running sum(mu^2) for the router)
                    mu_sb = []
                    for s in range(NS):
                        m = mu_pool.tile([128, Z], f32, tag="mu_sb")
                        nc.scalar.activation(m, pmu[s], AF.Copy)
                        a_mu2 = None
                        if nexp > 1:
                            sq = work.tile([128, Z], f32, tag="scratch")
                            a_mu2 = small.tile([128, 1], f32, tag="a_mu2")
                            nc.scalar.activation(sq, pmu[s], AF.Square,
                                                 scale=0.7071067811865476,
                                                 accum_out=a_mu2)
                        mu_sb.append((m, a_mu2))
                    pls = [ps.tile([128, Z], f32, tag="ps", name=f"pls_{tag}_{e}_{i}") for i in range(NS)]
                    for hc in range(HC):
                        w = wr_pool.tile([128, Z], wr_dt, tag="wr")
                        if nexp > 1:
                            src = wr_d[e, hc * 128:(hc + 1) * 128, Z:2 * Z]
                        else:
                            src = wr_d[hc * 128:(hc + 1) * 128, Z:2 * Z]
                        nc.sync.dma_start(out=w, in_=src)
                        for s in range(NS):
                            nc.tensor.matmul(
                                pls[s], hT[:, hc, s * 128:(s + 1) * 128], w,
                                start=(hc == 0), stop=(hc == HC - 1))
                    for s in range(NS):
                        mu, a_mu2 = mu_sb[s]
                        ex = work.tile([128, Z], f32, tag="ex")
                        nc.scalar.activation(ex, pls[s], AF.Exp)
                        if nexp > 1:
                            a_ls = small.tile([128, 1], f32, tag="a_ls")
                            sc1 = work.tile([128, Z], f32, tag="scratch")
                            nc.scalar.activation(sc1, pls[s], AF.Copy,
                                                 accum_out=a_ls)
                            a_e2 = small.tile([128, 1], f32, tag="a_e2")
                            sc2 = work.tile([128, Z], f32, tag="scratch")
                            nc.scalar.activation(sc2, ex, AF.Square,
                                                 scale=0.7071067811865476,
                                                 accum_out=a_e2)
                            # kl' = a_mu2 + a_e2 - 2 a_ls (argmax-equivalent)
                            t1 = small.tile([128, 1], f32, tag="t1")
                            nc.vector.tensor_add(t1, a_mu2, a_e2)
                            nc.vector.tensor_scalar(
                                out=kls[s][:, e:e + 1], in0=a_ls,
                                scalar1=-1.0, scalar2=t1,
                                op0=OP.mult, op1=OP.add)
                        exeps = work.tile([128, Z], f32, tag="exeps")
                        nc.vector.tensor_mul(exeps, ex, eps_t[s])
                        zt = zc_pool.tile([128, Z], f32, tag=f"z{e}_{s}",
                                          name=f"z_{tag}_{e}_{s}")
                        nc.vector.tensor_add(zt, mu, exeps)
                        z_cand[(e, s)] = zt

            # phase B/C: winner mask, masked transpose into zmT
            zmT = [zmt_pool.tile([128, ZC, BL], wc_dt, tag=f"zmT{i}", name=f"zmT_{tag}_{i}")
                   for i in range(nexp)]
            with scope(f"{tag}_select"):
                for s in range(NS):
                    fw = None
                    if nexp > 1:
                        m = small.tile([128, 1], f32, tag="klmax")
                        nc.vector.reduce_max(m, kls[s], axis=AX.X)
                        eq = small.tile([128, E], f32, tag="eq")
                        nc.vector.tensor_scalar(
                            out=eq, in0=kls[s], scalar1=m, scalar2=None,
                            op0=OP.is_equal)
                        # first-index tie-break (matches jnp.argmax): the
                        # first winner has the largest eq*rev8 = 8-e > 0.
                        tsel = small.tile([128, E], f32, tag="tsel")
                        nc.vector.tensor_mul(tsel, eq, rev8)
                        m2 = small.tile([128, 1], f32, tag="klmax2")
                        nc.vector.reduce_max(m2, tsel, axis=AX.X)
                        fw = small.tile([128, E], f32, tag="fw")
                        nc.vector.tensor_scalar(
                            out=fw, in0=tsel, scalar1=m2, scalar2=None,
                            op0=OP.is_equal)
                    for e in range(nexp):
                        zs = z_cand[(e, s)]
                        if nexp > 1:
                            dg = work.tile([128, 128], f32, tag="diag")
                            nc.vector.tensor_scalar(
                                out=dg, in0=ident, scalar1=fw[:, e:e + 1],
                                scalar2=None, op0=OP.mult)
                        else:
                            dg = ident
                        for zc in range(ZC):
                            pt = ps.tile([128, 128], f32, tag="ps")
                            nc.tensor.transpose(
                                pt, zs[:, zc * 128:(zc + 1) * 128], dg)
                            nc.scalar.activation(
                                zmT[e][:, zc, s * 128:(s + 1) * 128], pt,
                                AF.Copy)

            # phase D: h += sum_e zm_e @ Wc[e]
            with scope(f"{tag}_expert_out"):
                for hg in range(HC // 4):
                    ph = [ps.tile([128, 512], f32, tag="ps", name=f"ph_{tag}_{hg}_{i}") for i in range(4)]
                    for e in range(nexp):
                        for zc in range(ZC):
                            w = wc_pool.tile([128, 512], wc_dt, tag="wc")
                            if nexp > 1:
                                src = wc_d[e, zc * 128:(zc + 1) * 128,
                                           hg * 512:(hg + 1) * 512]
                            else:
                                src = wc_d[zc * 128:(zc + 1) * 128,
                                           hg * 512:(hg + 1) * 512]
                            nc.sync.dma_start(out=w, in_=src)
                            for hi in range(4):
                                nc.tensor.matmul(
                                    ph[hi], w[:, hi * 128:(hi + 1) * 128],
                                    zmT[e][:, zc, :],
                                    start=(e == 0 and zc == 0),
                                    stop=(e == nexp - 1 and zc == ZC - 1))
                    for hi in range(4):
                        hc = hg * 4 + hi
                        nc.vector.tensor_add(hT[:, hc, :], hT[:, hc, :], ph[hi])

        step(wr_enc, wc_enc, eps_d["enc"], 1, "enc")
        step(wr_comp, wc_comp, eps_d["c1"], E, "c1")
        step(wr_comp, wc_comp, eps_d["c2"], E, "c2")
        step(wr_dec, wc_dec, eps_d["dec"], 1, "dec")

        # ---- store h feature-major (host transposes back) ----------------
        with scope("store"):
            nc.sync.dma_start(
                out=out_d.rearrange("(c p) s -> p c s", p=128), in_=hT)

    _split_multiwait(nc, limit=1)
    return nc


def get_program(named_scopes=False):
    key = (WR_DTYPE, WC_DTYPE, named_scopes)
    if key not in _CACHE:
        _CACHE[key] = _build(named_scopes=named_scopes)
    return _CACHE[key]


def make_in_maps(inputs):
    import ml_dtypes
    npmap = {"f32": np.float32, "f32r": np.float32, "bf16": ml_dtypes.bfloat16}
    wr_np = npmap[WR_DTYPE]
    wc_np = npmap[WC_DTYPE]
    xT = np.asarray(inputs["x"], dtype=np.float32).T.astype(wr_np)  # [H, B]
    shared = {
        "wr_enc": np.ascontiguousarray(np.asarray(inputs["Wr_enc"][0]).astype(wr_np)),
        "wc_enc": np.ascontiguousarray(np.asarray(inputs["Wc_enc"][0]).astype(wc_np)),
        "wr_comp": np.ascontiguousarray(np.asarray(inputs["Wr_comp"]).astype(wr_np)),
        "wc_comp": np.ascontiguousarray(np.asarray(inputs["Wc_comp"]).astype(wc_np)),
        "wr_dec": np.ascontiguousarray(np.asarray(inputs["Wr_dec"][0]).astype(wr_np)),
        "wc_dec": np.ascontiguousarray(np.asarray(inputs["Wc_dec"][0]).astype(wc_np)),
        "rev8": np.tile(np.arange(E, 0, -1, dtype=np.float32), (128, 1)),
    }
    maps = []
    for c in range(NCORES):
        sl = slice(c * BL, (c + 1) * BL)
        m = dict(shared)
        m["x"] = np.ascontiguousarray(xT[:, sl])
        m["eps_enc"] = np.ascontiguousarray(inputs["eps_enc"][sl], np.float32)
        m["eps1"] = np.ascontiguousarray(inputs["eps1"][sl], np.float32)
        m["eps2"] = np.ascontiguousarray(inputs["eps2"][sl], np.float32)
        m["eps_dec"] = np.ascontiguousarray(inputs["eps_dec"][sl], np.float32)
        maps.append(m)
    return maps


def kernel(**inputs):
    from concourse.bass_utils import run_bass_kernel_spmd

    nc = get_program()
    res = run_bass_kernel_spmd(nc, make_in_maps(inputs),
                               core_ids=list(range(NCORES)))
    out = np.concatenate(
        [np.asarray(res.results[c]["out"]).astype(np.float32).T
         for c in range(NCORES)], axis=0)
    return out
